# revision 62
# baseline (speedup 1.0000x reference)
"""MoE (gating + 8 experts, BN-folded) Trainium2 Bass kernel, v4.

Contract: kernel(**inputs) takes the FULL unsharded inputs (numpy, keyed as in
setup_inputs()) and returns the FULL [65536, 1] float32 output.

Strategy (v4, evolved from v3; ~226us vs v3's 267.8us on core 0):
  * Data-parallel over 8 NeuronCores: batch 65536 -> 8192 rows per core.
  * All BatchNorms folded into the adjacent Linear weights/biases on host.
  * Activations live as [features(partitions), batch(free)]; x is transposed
    host-side per shard and stays resident in SBUF.
  * Software pipeline with a full step of slack between producer and consumer:
    step t runs L0(t) and L1(t-1), so every PSUM evacuation has ~3us before
    its result is needed and the PE sustains the 216 ns/matmul back-to-back
    rate (v3 ran L1(t) in step t and stalled on the evac turnaround).
  * L2 runs as 2-tile bursts on even steps (tiles t-3 and t-2), each tile a
    pair of col-tiled M=64 matmuls (tile_position (0,0)/(0,64)) running
    concurrently on separate array column strips. A col-tiled group pays
    ~217ns of entry/exit transitions against the surrounding full-array
    stream (strip LDWEIGHTS cannot prefetch under a full-row matmul, and the
    strip drain delays the next full-array matmul), so bursts amortize it.
  * Zproj batched per 4 tiles as a 4x col-tiled M=32 burst into one PSUM bank
    at t=4q+7 (strip s = tile 4q+s, rows 32s..32s+1 hold z_a/z_b), one DVE
    evac + one whole-tile DMA per burst.
  * gating runs at t=12/13 with a 4x col-tiled M=32 logit burst at t=15, so
    its export is never on the kernel-end critical path.
  * Evacuation assignment (h0mc0/h1a/zb/gh on DVE; h0mc1/h1b/h2/eg on ACT) is
    tuned so each engine's queue order matches its dependency-fire order;
    per-engine FIFO head-of-line blocking otherwise stalls the PE rings.
  * PSUM budget: p0 4 banks (L0), p1 2 banks (L1), shared 2-bank ring
    (L2 / gating / Z). 8 cold warm-up matmuls span the HAM clock ramp.
  * z and raw gate numerators exp(logits) are exported; the host computes
    y = sum_e g_e z_e / sum_e g_e + ob in float64.
"""

import numpy as np
import ml_dtypes

EPS = 1e-5
B, D, E, G, H0, H1, H2 = 65536, 256, 8, 128, 256, 128, 64
NCORES = 8
NB = B // NCORES          # rows per core
TB = 512                  # batch tile (matmul free dim / PSUM bank)
NT = NB // TB             # batch tiles per core
KD = D // 128             # k-chunks over D
NPAIR = E // 2


def _fold_params(inputs):
    """Fold the four BatchNorms into the adjacent Linears. float64 math."""
    f = {k: np.asarray(v, dtype=np.float64) for k, v in inputs.items()}

    s_in = f["in_g"] / np.sqrt(f["in_v"] + EPS)            # [D]
    t_in = f["in_b"] - f["in_m"] * s_in                    # [D]

    # gating L1 (+input BN folded in)
    a_g = f["g_g"] / np.sqrt(f["g_v"] + EPS)               # [G]
    w1 = f["gW1"] * a_g[None, :]                           # [D,G]
    W1f = s_in[:, None] * w1
    b1f = t_in @ w1 + (f["gb1"] - f["g_m"]) * a_g + f["g_b"]

    # expert L0 (+input BN)
    a0 = f["e0g"] / np.sqrt(f["e0v"] + EPS)                # [E,H0]
    w0 = f["eW0"] * a0[:, None, :]                         # [E,D,H0]
    W0f = s_in[None, :, None] * w0
    b0f = np.einsum("d,edo->eo", t_in, w0) + (f["eb0"] - f["e0m"]) * a0 + f["e0b"]

    a1 = f["e1g"] / np.sqrt(f["e1v"] + EPS)
    W1ef = f["eW1"] * a1[:, None, :]                       # [E,H0,H1]
    b1ef = (f["eb1"] - f["e1m"]) * a1 + f["e1b"]

    a2 = f["e2g"] / np.sqrt(f["e2v"] + EPS)
    W2f = f["eW2"] * a2[:, None, :]                        # [E,H1,H2]
    b2f = (f["eb2"] - f["e2m"]) * a2 + f["e2b"]

    g32 = lambda a: np.ascontiguousarray(a, dtype=np.float32)
    gbf = lambda a: np.ascontiguousarray(a, dtype=np.float32).astype(ml_dtypes.bfloat16)

    dev = {}
    dev["WG1"] = gbf(W1f.reshape(KD, 128, G).transpose(1, 0, 2))          # [128,KD,G]
    dev["BG1"] = g32(b1f[:, None])                                        # [G,1]
    WG2 = np.zeros((G, 32), dtype=np.float64)
    WG2[:, 0:E] = f["gW2"]                                                # M=32 strip
    dev["WG2"] = gbf(WG2)                                                 # [128,32]
    BG2 = np.zeros((128, 1), dtype=np.float64)
    for s in range(4):
        BG2[32 * s:32 * s + E, 0] = f["gb2"]
    dev["BG2"] = g32(BG2)                                                 # [128,1]
    dev["WE0"] = gbf(W0f.reshape(E, KD, 128, 2, 128).transpose(2, 0, 1, 3, 4))  # [128,E,KD,2,128]
    dev["BE0"] = g32(b0f.reshape(E, 2, 128).transpose(2, 0, 1))           # [128,E,2]
    dev["WE1"] = gbf(W1ef.reshape(E, 2, 128, H1).transpose(2, 0, 1, 3))   # [128,E,2,H1]
    dev["BE1"] = g32(b1ef.T)                                              # [H1,E]
    WE2 = np.zeros((128, NPAIR, 2, 64), dtype=np.float64)                 # per-ex strips
    BE2 = np.zeros((128, NPAIR), dtype=np.float64)
    for j in range(NPAIR):
        WE2[:, j, 0, :] = W2f[2 * j]                                      # [H1,64]
        WE2[:, j, 1, :] = W2f[2 * j + 1]
        BE2[0:64, j] = b2f[2 * j]
        BE2[64:128, j] = b2f[2 * j + 1]
    dev["WE2"] = gbf(WE2)
    dev["BE2"] = g32(BE2)
    ow = f["oW"][:, 0]                                                    # [H2]
    OWZ = np.zeros((128, 32), dtype=np.float64)                           # M=32 strip
    OWZ[0:64, 0] = ow
    OWZ[64:128, 1] = ow
    dev["OWZ"] = gbf(OWZ)
    ob = float(f["ob"][0])
    return dev, ob


def _build_program(fast_h0=False):
    """fast_h0: h0 biases are all zero (true for the reference init, where
    every BatchNorm is identity and every Linear bias is zero), so the two
    per-mc h0 evacuations merge into one FD=1024 two-bank op per expert.
    The general path (per-mc bias via tensor_scalar/activation) is kept as
    the fallback for nonzero biases."""
    import concourse.bass as bass
    import concourse.mybir as mybir
    import concourse.tile as tile
    from concourse import bacc

    f32 = mybir.dt.float32
    bf16 = mybir.dt.bfloat16
    Relu = mybir.ActivationFunctionType.Relu
    Exp = mybir.ActivationFunctionType.Exp
    Copy = mybir.ActivationFunctionType.Copy
    add = mybir.AluOpType.add
    amax = mybir.AluOpType.max

    nc = bacc.Bacc("TRN2", target_bir_lowering=False, debug=False)

    xT = nc.dram_tensor("xT", [D, NB], bf16, kind="ExternalInput").ap()
    # zs[j, q, 32s+i, col] = z for expert 2j+i, batch tile 4q+s (full zb tile,
    # one large DMA per burst beats 4 small ones: each DMA_DIRECT2D costs
    # ~600ns on its issuing queue and the tail ones serialized at kernel end)
    zs = nc.dram_tensor("zs", [NPAIR, NT // 4, 128, TB], f32, kind="ExternalOutput").ap()
    # eg[j, 32s+e, col] = exp(logit_e) for batch tile 4j+s
    eg = nc.dram_tensor("eg", [NPAIR, 128, TB], f32, kind="ExternalOutput").ap()

    d_in = {}
    shapes = {
        "WG1": ([128, KD, G], bf16), "BG1": ([G, 1], f32),
        "WG2": ([128, 32], bf16), "BG2": ([128, 1], f32),
        "WE0": ([128, E, KD, 2, 128], bf16), "BE0": ([128, E, 2], f32),
        "WE1": ([128, E, 2, H1], bf16), "BE1": ([H1, E], f32),
        "WE2": ([128, NPAIR, 2, 64], bf16), "BE2": ([128, NPAIR], f32),
        "OWZ": ([128, 32], bf16),
    }
    for name, (shape, dt) in shapes.items():
        d_in[name] = nc.dram_tensor(name, shape, dt, kind="ExternalInput").ap()

    with tile.TileContext(nc) as tc:
        with (
            tc.tile_pool(name="consts", bufs=1) as consts,
            tc.tile_pool(name="xp", bufs=NT) as xp,
            tc.tile_pool(name="ghp", bufs=5) as ghp,
            tc.tile_pool(name="egp", bufs=2) as egp,
            tc.tile_pool(name="h0p", bufs=6) as h0p,
            tc.tile_pool(name="h1p", bufs=8) as h1p,
            tc.tile_pool(name="h2p", bufs=7) as h2p,
            tc.tile_pool(name="zbp", bufs=2) as zbp,
            tc.tile_pool(name="p0", bufs=(2 if fast_h0 else 4),
                         space="PSUM") as p0,                        # 4 banks: L0
            tc.tile_pool(name="p1", bufs=2, space="PSUM") as p1,     # 2 banks: L1
            tc.tile_pool(name="p2", bufs=2, space="PSUM") as p2,     # 2 banks: L2/gate/Z
        ):
            W = {}
            for name, ap in d_in.items():
                W[name] = consts.tile(list(ap.shape), shapes[name][1], tag=name,
                                      name=name)
            # Expert weights stream on the gpsimd queue in usage order; the
            # small biases ride the scalar queue so nothing serializes behind
            # the big transfers.
            def load_pair(j):
                # both experts' L0 weights first: the first L0 step needs
                # WE0 of BOTH experts before any WE1 is touched
                for i in (0, 1):
                    e = 2 * j + i
                    nc.gpsimd.dma_start(W["WE0"][:, e], d_in["WE0"][:, e])
                for i in (0, 1):
                    e = 2 * j + i
                    nc.gpsimd.dma_start(W["WE1"][:, e], d_in["WE1"][:, e])
                nc.gpsimd.dma_start(W["WE2"][:, j], d_in["WE2"][:, j])

            # warm-tile memset FIRST on the gpsimd queue (before load_pair's
            # DMAs land on it) so the PE warm-up can start as soon as the
            # engines leave the NEFF preamble.
            warm = consts.tile([128, TB], bf16, tag="warm", name="warm")
            nc.gpsimd.memset(warm[:], 0.0)

            load_pair(0)
            for name in ("BE0", "BE1", "WG1", "BG1", "BE2", "WG2", "BG2", "OWZ"):
                nc.scalar.dma_start(W[name][:], d_in[name][:])

            # first 4 x tiles up-front; the rest stream during phase 0 so the
            # early DMA bandwidth goes to the pair-0 weights.
            xt = []

            def load_x(t):
                bs = t * TB
                xti = xp.tile([128, KD, TB], bf16, tag="xt", name=f"x{t}")
                for c in range(KD):
                    # tile 0 splits its chunks across two queues so the very
                    # first L0 step isn't serialized behind one DMA queue
                    eng = nc.scalar if (t == 0 and c == 1) else nc.sync
                    eng.dma_start(xti[:, c, :], xT[c * 128:(c + 1) * 128, bs:bs + TB])
                xt.append(xti)

            for t in range(4):
                load_x(t)

            # PE warm-up: the HAM clock gate needs ~3.4us of sustained matmul
            # activity to lift the PE from 1.2 to 2.4 GHz; burn the DMA fill
            # time on dummy matmuls over a zeroed tile so the real matmuls
            # start warm.
            # PE warm-up: 8 cold matmuls (427ns each) span exactly the ~3.4us
            # HAM clock-gate ramp, so the first real L0 runs at 2.4 GHz.
            # (warm=3 measured slower: the cold-clock L0s delay their evacs
            # and the stall ripples through the pipeline.)
            for k in range(8):
                pool = (p1, p2)[k % 2]
                tag = "mm1" if k % 2 == 0 else "mm2"
                psw = pool.tile([128, TB], f32, tag=tag, name=f"warm{k}")
                nc.tensor.matmul(psw[:], warm[:, 0:128], warm[:],
                                 start=True, stop=True)

            def dve_evac(dst, src, bias_ap):
                nc.vector.tensor_scalar(out=dst, in0=src, scalar1=bias_ap,
                                        scalar2=0.0, op0=add, op1=amax)

            # ---- expert pair phases ----
            for j in range(NPAIR):
                ea, eb = 2 * j, 2 * j + 1
                h0t = {}
                h1t = {}
                h2t = {}
                ghq = {}

                def l1_block(tm, i, ex):
                    """L1 for tile tm, expert slot i. Both h1 evacs ride DVE:
                    the ACT queue drifts ~a step behind (its h2 op depends on
                    the late-in-step L2), which made ACT-side h1 arrive
                    just-in-time for L2 and stall its strip LDW."""
                    ps1 = p1.tile([128, TB], f32, tag="mm1", name=f"ps1_{i}")
                    h0prev = h0t[tm][i]
                    for c in range(2):
                        nc.tensor.matmul(ps1[:], W["WE1"][:, ex, c, :],
                                         h0prev[:, c, :], start=(c == 0),
                                         stop=(c == 1))
                    h1_ = h1p.tile([128, TB], bf16, tag="h1", name=f"h1_{i}")
                    if i == 0:
                        dve_evac(h1_[:], ps1[:], W["BE1"][:, ex:ex + 1])
                    else:
                        nc.scalar.activation(h1_[:], ps1[:], Relu,
                                             bias=W["BE1"][:, ex:ex + 1])
                    h1t.setdefault(tm, [None, None])[i] = h1_

                def gating_a(q):
                    psg = p2.tile([128, TB], f32, tag="mm2", name="psg")
                    for c in range(KD):
                        nc.tensor.matmul(psg[:], W["WG1"][:, c, :],
                                         xt[4 * j + q][:, c, :],
                                         start=(c == 0), stop=(c == KD - 1))
                    gh_ = ghp.tile([128, TB], bf16, tag="gh", name=f"gh{q}")
                    dve_evac(gh_[:], psg[:], W["BG1"][:, 0:1])
                    ghq[q] = gh_

                for t in range(NT + 5):
                    if j == 0 and 1 <= t <= 12:
                        load_x(t + 3)
                        if t % 4 == 0:
                            load_pair(t // 4)
                    tm1, tm2, tm3 = t - 1, t - 2, t - 3

                    # gating runs late in each phase (t=12/13, burst 15) so
                    # the eg export is off the kernel-end critical path
                    ga_t = 12
                    if t == ga_t:
                        gating_a(0)
                        gating_a(1)
                    if t == ga_t + 1:
                        gating_a(2)
                        gating_a(3)

                    # L0 for tile t (both experts), L1 for tile t-1 interleaved
                    if t < NT:
                        h0 = [h0p.tile([128, 2, TB], bf16, tag="h0",
                                       name=f"h0_{i}") for i in (0, 1)]
                        for i, ex in ((0, ea), (1, eb)):
                            if fast_h0:
                                ps0 = p0.tile([128, 2, TB], f32, tag="mm0",
                                              name=f"ps0_{i}")
                                for mc in range(2):
                                    for c in range(KD):
                                        nc.tensor.matmul(ps0[:, mc, :],
                                                         W["WE0"][:, ex, c, mc, :],
                                                         xt[t][:, c, :],
                                                         start=(c == 0),
                                                         stop=(c == KD - 1))
                                if i == 0:
                                    nc.vector.tensor_scalar_max(h0[i][:], ps0[:],
                                                                0.0)
                                else:
                                    nc.scalar.activation(h0[i][:], ps0[:], Relu)
                            else:
                                for mc in range(2):
                                    ps0 = p0.tile([128, TB], f32, tag="mm0",
                                                  name=f"ps0_{i}{mc}")
                                    for c in range(KD):
                                        nc.tensor.matmul(ps0[:],
                                                         W["WE0"][:, ex, c, mc, :],
                                                         xt[t][:, c, :],
                                                         start=(c == 0),
                                                         stop=(c == KD - 1))
                                    if mc == 0:
                                        dve_evac(h0[i][:, 0, :], ps0[:],
                                                 W["BE0"][:, ex, 0:1])
                                    else:
                                        nc.scalar.activation(h0[i][:, mc, :],
                                                             ps0[:], Relu,
                                                             bias=W["BE0"][:, ex, mc:mc + 1])
                            if tm1 >= 0:
                                l1_block(tm1, i, ex)
                        h0t[t] = h0
                        if tm1 >= 0:
                            del h0t[tm1]
                    elif t == NT:
                        for i, ex in ((0, ea), (1, eb)):
                            l1_block(tm1, i, ex)
                        del h0t[tm1]

                    # L2 for tiles (t-3, t-2) batched on even steps: the
                    # col-tiled group's entry (strip LDW can't prefetch under
                    # a full-row matmul) and exit (strip drain delays the next
                    # full-array matmul) costs ~217ns and is paid once per
                    # burst instead of once per tile.
                    # even steps cover (t-3, t-2); the extra t=17 slot pulls
                    # tile 15 one step earlier to shorten the drain tail
                    if t % 2 == 0 or t == 17:
                        for tl in (t - 3, t - 2):
                            if not (0 <= tl < NT) or tl not in h1t:
                                continue
                            ps2 = p2.tile([128, TB], f32, tag="mm2", name="ps2")
                            ha, hb = h1t[tl]
                            nc.tensor.matmul(ps2[0:64, :], W["WE2"][:, j, 0, :],
                                             ha[:], start=True, stop=True,
                                             tile_position=(0, 0),
                                             skip_group_check=True)
                            nc.tensor.matmul(ps2[64:128, :], W["WE2"][:, j, 1, :],
                                             hb[:], start=True, stop=True,
                                             tile_position=(0, 64),
                                             skip_group_check=True)
                            h2_ = h2p.tile([128, TB], bf16, tag="h2", name="h2")
                            nc.scalar.activation(h2_[:], ps2[:], Relu,
                                                 bias=W["BE2"][:, j:j + 1])
                            h2t[tl] = h2_
                            del h1t[tl]

                    # Z burst for tiles 4q..4q+3 at t=4q+7 (odd steps, away
                    # from the even-step L2 bursts; clustering them measured
                    # slower - the combined step overloads the evac engines).
                    # The last group fires at t=18 (h2(15) lands at t=17).
                    zq = None
                    if t in (7, 11, 15):
                        zq = (t - 7) // 4
                    elif t == 18:
                        zq = 3
                    if zq is not None:
                        q = zq
                        psz = p2.tile([128, TB], f32, tag="mm2", name="psz")
                        for s in range(4):
                            nc.tensor.matmul(psz[32 * s:32 * s + 32, :],
                                             W["OWZ"][:], h2t[4 * q + s][:],
                                             start=True, stop=True,
                                             tile_position=(0, 32 * s),
                                             skip_group_check=True)
                        zb = zbp.tile([128, TB], f32, tag="zb", name="zb")
                        nc.vector.tensor_scalar_add(zb[:], psz[:], 0.0)
                        if j == NPAIR - 1 and q == NT // 4 - 1:
                            # final export: 4 tiny row-pair DMAs spread over 4
                            # queues land ~3us earlier than one 256KB transfer
                            # keep gpsimd out of the final exports: its
                            # teardown DRAIN waits on the whole DMA ring
                            for s, eng in ((0, nc.sync), (1, nc.scalar),
                                           (2, nc.sync), (3, nc.scalar)):
                                eng.dma_start(zs[j, q, 32 * s:32 * s + 2],
                                              zb[32 * s:32 * s + 2, :])
                        else:
                            nc.sync.dma_start(zs[j, q], zb[:])
                        for s in range(4):
                            del h2t[4 * q + s]

                    # gating L2 burst: 4 col-tiled M=32 (strip s = tile 4j+s)
                    if t == 15:
                        psb = p2.tile([128, TB], f32, tag="mm2", name="psb")
                        for s in range(4):
                            nc.tensor.matmul(psb[32 * s:32 * s + 32, :],
                                             W["WG2"][:], ghq[s][:],
                                             start=True, stop=True,
                                             tile_position=(0, 32 * s),
                                             skip_group_check=True)
                        egb = egp.tile([128, TB], f32, tag="eg", name="egb")
                        nc.scalar.activation(egb[:], psb[:], Exp,
                                             bias=W["BG2"][:, 0:1])
                        nc.gpsimd.dma_start(eg[j], egb[:])

    nc.compile()
    return nc


_CACHE = {}


def _get_program(fast_h0):
    key = ("nc", fast_h0)
    if key not in _CACHE:
        _CACHE[key] = _build_program(fast_h0=fast_h0)
    return _CACHE[key]


def _run(inputs, trace=False):
    from concourse.bass_utils import run_bass_kernel_spmd

    x = np.ascontiguousarray(np.asarray(inputs["x"], dtype=np.float32))
    dev, ob = _fold_params(inputs)
    # fast_h0 measured SLOWER on hw (251us vs 229us): the merged FD=1024 evac
    # releases the two-bank p0 ring later and stalls the next step's L0.
    fast_h0 = False
    nc = _get_program(fast_h0)

    in_maps = []
    for c in range(NCORES):
        m = dict(dev)
        xs = np.ascontiguousarray(x[c * NB:(c + 1) * NB, :].T)
        m["xT"] = xs.astype(ml_dtypes.bfloat16)
        in_maps.append(m)

    kwargs = {}
    if trace:
        kwargs = dict(trace=True, trace_cores=[0])
    res = run_bass_kernel_spmd(nc, in_maps, core_ids=list(range(NCORES)), **kwargs)

    outs = []
    for c in range(NCORES):
        zf = res.results[c]["zs"].astype(np.float64)     # [NPAIR, NT//4, 128, TB]
        gf = res.results[c]["eg"].astype(np.float64)     # [NPAIR, 128, TB]
        # zf[j, q, 32s+i] -> [t=4q+s, e=2j+i, col]
        z = zf.reshape(NPAIR, NT // 4, 4, 32, TB)[:, :, :, 0:2]
        z2 = z.transpose(1, 2, 0, 3, 4).reshape(NT, E, TB)
        # gf[j, 32s+e]: gating tile t=4j+s -> [t, e, col]
        g = gf.reshape(NPAIR, 4, 32, TB)[:, :, 0:E]
        g2 = g.reshape(NT, E, TB)
        num = np.sum(g2 * z2, axis=1)                    # [NT, TB]
        den = np.sum(g2, axis=1)
        outs.append((num / den).reshape(-1))
    out = np.concatenate(outs) + ob
    return out.astype(np.float32)[:, None], res


def kernel(**inputs):
    out, _ = _run(inputs, trace=False)
    return out


def kernel_traced(**inputs):
    return _run(inputs, trace=True)


# revision 63
# speedup vs baseline: 1.0303x; 1.0303x over previous
"""MoE (gating + 8 experts, BN-folded) Trainium2 Bass kernel, v4.

Contract: kernel(**inputs) takes the FULL unsharded inputs (numpy, keyed as in
setup_inputs()) and returns the FULL [65536, 1] float32 output.

Strategy (v4, evolved from v3; ~226us vs v3's 267.8us on core 0):
  * Data-parallel over 8 NeuronCores: batch 65536 -> 8192 rows per core.
  * All BatchNorms folded into the adjacent Linear weights/biases on host.
  * Activations live as [features(partitions), batch(free)]; x is transposed
    host-side per shard and stays resident in SBUF.
  * Software pipeline with a full step of slack between producer and consumer:
    step t runs L0(t) and L1(t-1), so every PSUM evacuation has ~3us before
    its result is needed and the PE sustains the 216 ns/matmul back-to-back
    rate (v3 ran L1(t) in step t and stalled on the evac turnaround).
  * L2 runs as 2-tile bursts on even steps (tiles t-3 and t-2), each tile a
    pair of col-tiled M=64 matmuls (tile_position (0,0)/(0,64)) running
    concurrently on separate array column strips. A col-tiled group pays
    ~217ns of entry/exit transitions against the surrounding full-array
    stream (strip LDWEIGHTS cannot prefetch under a full-row matmul, and the
    strip drain delays the next full-array matmul), so bursts amortize it.
  * Zproj batched per 4 tiles as a 4x col-tiled M=32 burst into one PSUM bank
    at t=4q+7 (strip s = tile 4q+s, rows 32s..32s+1 hold z_a/z_b), one DVE
    evac + one whole-tile DMA per burst.
  * gating runs at t=12/13 with a 4x col-tiled M=32 logit burst at t=15, so
    its export is never on the kernel-end critical path.
  * Evacuation assignment (h0mc0/h1a/zb/gh on DVE; h0mc1/h1b/h2/eg on ACT) is
    tuned so each engine's queue order matches its dependency-fire order;
    per-engine FIFO head-of-line blocking otherwise stalls the PE rings.
  * PSUM budget: p0 4 banks (L0), p1 2 banks (L1), shared 2-bank ring
    (L2 / gating / Z). 8 cold warm-up matmuls span the HAM clock ramp.
  * z and raw gate numerators exp(logits) are exported; the host computes
    y = sum_e g_e z_e / sum_e g_e + ob in float64.
"""

import numpy as np
import ml_dtypes

EPS = 1e-5
B, D, E, G, H0, H1, H2 = 65536, 256, 8, 128, 256, 128, 64
NCORES = 8
NB = B // NCORES          # rows per core
TB = 512                  # batch tile (matmul free dim / PSUM bank)
NT = NB // TB             # batch tiles per core
KD = D // 128             # k-chunks over D
NPAIR = E // 2


def _fold_params(inputs):
    """Fold the four BatchNorms into the adjacent Linears. float64 math."""
    f = {k: np.asarray(v, dtype=np.float64) for k, v in inputs.items()}

    s_in = f["in_g"] / np.sqrt(f["in_v"] + EPS)            # [D]
    t_in = f["in_b"] - f["in_m"] * s_in                    # [D]

    # gating L1 (+input BN folded in)
    a_g = f["g_g"] / np.sqrt(f["g_v"] + EPS)               # [G]
    w1 = f["gW1"] * a_g[None, :]                           # [D,G]
    W1f = s_in[:, None] * w1
    b1f = t_in @ w1 + (f["gb1"] - f["g_m"]) * a_g + f["g_b"]

    # expert L0 (+input BN)
    a0 = f["e0g"] / np.sqrt(f["e0v"] + EPS)                # [E,H0]
    w0 = f["eW0"] * a0[:, None, :]                         # [E,D,H0]
    W0f = s_in[None, :, None] * w0
    b0f = np.einsum("d,edo->eo", t_in, w0) + (f["eb0"] - f["e0m"]) * a0 + f["e0b"]

    a1 = f["e1g"] / np.sqrt(f["e1v"] + EPS)
    W1ef = f["eW1"] * a1[:, None, :]                       # [E,H0,H1]
    b1ef = (f["eb1"] - f["e1m"]) * a1 + f["e1b"]

    a2 = f["e2g"] / np.sqrt(f["e2v"] + EPS)
    W2f = f["eW2"] * a2[:, None, :]                        # [E,H1,H2]
    b2f = (f["eb2"] - f["e2m"]) * a2 + f["e2b"]

    g32 = lambda a: np.ascontiguousarray(a, dtype=np.float32)
    gbf = lambda a: np.ascontiguousarray(a, dtype=np.float32).astype(ml_dtypes.bfloat16)

    dev = {}
    dev["WG1"] = gbf(W1f.reshape(KD, 128, G).transpose(1, 0, 2))          # [128,KD,G]
    dev["BG1"] = g32(b1f[:, None])                                        # [G,1]
    WG2 = np.zeros((G, 32), dtype=np.float64)
    WG2[:, 0:E] = f["gW2"]                                                # M=32 strip
    dev["WG2"] = gbf(WG2)                                                 # [128,32]
    BG2 = np.zeros((128, 1), dtype=np.float64)
    for s in range(4):
        BG2[32 * s:32 * s + E, 0] = f["gb2"]
    dev["BG2"] = g32(BG2)                                                 # [128,1]
    dev["WE0"] = gbf(W0f.reshape(E, KD, 128, 2, 128).transpose(2, 0, 1, 3, 4))  # [128,E,KD,2,128]
    dev["BE0"] = g32(b0f.reshape(E, 2, 128).transpose(2, 0, 1))           # [128,E,2]
    dev["WE1"] = gbf(W1ef.reshape(E, 2, 128, H1).transpose(2, 0, 1, 3))   # [128,E,2,H1]
    dev["BE1"] = g32(b1ef.T)                                              # [H1,E]
    WE2 = np.zeros((128, NPAIR, 2, 64), dtype=np.float64)                 # per-ex strips
    BE2 = np.zeros((128, NPAIR), dtype=np.float64)
    for j in range(NPAIR):
        WE2[:, j, 0, :] = W2f[2 * j]                                      # [H1,64]
        WE2[:, j, 1, :] = W2f[2 * j + 1]
        BE2[0:64, j] = b2f[2 * j]
        BE2[64:128, j] = b2f[2 * j + 1]
    dev["WE2"] = gbf(WE2)
    dev["BE2"] = g32(BE2)
    ow = f["oW"][:, 0]                                                    # [H2]
    OWZ = np.zeros((128, 32), dtype=np.float64)                           # M=32 strip
    OWZ[0:64, 0] = ow
    OWZ[64:128, 1] = ow
    dev["OWZ"] = gbf(OWZ)
    ob = float(f["ob"][0])
    return dev, ob


def _build_program(fast_h0=False):
    """fast_h0: h0 biases are all zero (true for the reference init, where
    every BatchNorm is identity and every Linear bias is zero), so the two
    per-mc h0 evacuations merge into one FD=1024 two-bank op per expert.
    The general path (per-mc bias via tensor_scalar/activation) is kept as
    the fallback for nonzero biases."""
    import concourse.bass as bass
    import concourse.mybir as mybir
    import concourse.tile as tile
    from concourse import bacc

    f32 = mybir.dt.float32
    bf16 = mybir.dt.bfloat16
    Relu = mybir.ActivationFunctionType.Relu
    Exp = mybir.ActivationFunctionType.Exp
    Copy = mybir.ActivationFunctionType.Copy
    add = mybir.AluOpType.add
    amax = mybir.AluOpType.max

    nc = bacc.Bacc("TRN2", target_bir_lowering=False, debug=False)

    xT = nc.dram_tensor("xT", [D, NB], bf16, kind="ExternalInput").ap()
    # zs[j, q, 32s+i, col] = z for expert 2j+i, batch tile 4q+s (full zb tile,
    # one large DMA per burst beats 4 small ones: each DMA_DIRECT2D costs
    # ~600ns on its issuing queue and the tail ones serialized at kernel end)
    zs = nc.dram_tensor("zs", [NPAIR, NT // 4, 128, TB], f32, kind="ExternalOutput").ap()
    # eg[j, 32s+e, col] = exp(logit_e) for batch tile 4j+s
    eg = nc.dram_tensor("eg", [NPAIR, 128, TB], f32, kind="ExternalOutput").ap()

    d_in = {}
    shapes = {
        "WG1": ([128, KD, G], bf16), "BG1": ([G, 1], f32),
        "WG2": ([128, 32], bf16), "BG2": ([128, 1], f32),
        "WE0": ([128, E, KD, 2, 128], bf16), "BE0": ([128, E, 2], f32),
        "WE1": ([128, E, 2, H1], bf16), "BE1": ([H1, E], f32),
        "WE2": ([128, NPAIR, 2, 64], bf16), "BE2": ([128, NPAIR], f32),
        "OWZ": ([128, 32], bf16),
    }
    for name, (shape, dt) in shapes.items():
        d_in[name] = nc.dram_tensor(name, shape, dt, kind="ExternalInput").ap()

    with tile.TileContext(nc) as tc:
        with (
            tc.tile_pool(name="consts", bufs=1) as consts,
            tc.tile_pool(name="xp", bufs=NT) as xp,
            tc.tile_pool(name="ghp", bufs=5) as ghp,
            tc.tile_pool(name="egp", bufs=2) as egp,
            tc.tile_pool(name="h0p", bufs=6) as h0p,
            tc.tile_pool(name="h1p", bufs=8) as h1p,
            tc.tile_pool(name="h2p", bufs=7) as h2p,
            tc.tile_pool(name="zbp", bufs=2) as zbp,
            tc.tile_pool(name="p0", bufs=(2 if fast_h0 else 4),
                         space="PSUM") as p0,                        # 4 banks: L0
            tc.tile_pool(name="p1", bufs=2, space="PSUM") as p1,     # 2 banks: L1
            tc.tile_pool(name="p2", bufs=2, space="PSUM") as p2,     # 2 banks: L2/gate/Z
        ):
            W = {}
            for name, ap in d_in.items():
                W[name] = consts.tile(list(ap.shape), shapes[name][1], tag=name,
                                      name=name)
            # Expert weights stream on the gpsimd queue in usage order; the
            # small biases ride the scalar queue so nothing serializes behind
            # the big transfers.
            def load_pair(j):
                # both experts' L0 weights first: the first L0 step needs
                # WE0 of BOTH experts before any WE1 is touched
                for i in (0, 1):
                    e = 2 * j + i
                    nc.gpsimd.dma_start(W["WE0"][:, e], d_in["WE0"][:, e])
                for i in (0, 1):
                    e = 2 * j + i
                    nc.gpsimd.dma_start(W["WE1"][:, e], d_in["WE1"][:, e])
                nc.gpsimd.dma_start(W["WE2"][:, j], d_in["WE2"][:, j])

            # warm-tile memset FIRST on the gpsimd queue (before load_pair's
            # DMAs land on it) so the PE warm-up can start as soon as the
            # engines leave the NEFF preamble.
            warm = consts.tile([128, TB], bf16, tag="warm", name="warm")
            nc.gpsimd.memset(warm[:], 0.0)

            load_pair(0)
            for name in ("BE0", "BE1", "WG1", "BG1", "BE2", "WG2", "BG2", "OWZ"):
                nc.scalar.dma_start(W[name][:], d_in[name][:])

            # first 4 x tiles up-front; the rest stream during phase 0 so the
            # early DMA bandwidth goes to the pair-0 weights.
            xt = []

            def load_x(t):
                bs = t * TB
                xti = xp.tile([128, KD, TB], bf16, tag="xt", name=f"x{t}")
                for c in range(KD):
                    nc.sync.dma_start(xti[:, c, :], xT[c * 128:(c + 1) * 128, bs:bs + TB])
                xt.append(xti)

            for t in range(4):
                load_x(t)

            # PE warm-up: the HAM clock gate needs ~3.4us of sustained matmul
            # activity to lift the PE from 1.2 to 2.4 GHz; burn the DMA fill
            # time on dummy matmuls over a zeroed tile so the real matmuls
            # start warm.
            # PE warm-up: 8 cold matmuls (427ns each) span exactly the ~3.4us
            # HAM clock-gate ramp, so the first real L0 runs at 2.4 GHz.
            # (warm=3 measured slower: the cold-clock L0s delay their evacs
            # and the stall ripples through the pipeline.)
            for k in range(8):
                pool = (p1, p2)[k % 2]
                tag = "mm1" if k % 2 == 0 else "mm2"
                psw = pool.tile([128, TB], f32, tag=tag, name=f"warm{k}")
                nc.tensor.matmul(psw[:], warm[:, 0:128], warm[:],
                                 start=True, stop=True)

            def dve_evac(dst, src, bias_ap):
                nc.vector.tensor_scalar(out=dst, in0=src, scalar1=bias_ap,
                                        scalar2=0.0, op0=add, op1=amax)

            # ---- expert pair phases ----
            for j in range(NPAIR):
                ea, eb = 2 * j, 2 * j + 1
                h0t = {}
                h1t = {}
                h2t = {}
                ghq = {}

                def l1_block(tm, i, ex):
                    """L1 for tile tm, expert slot i. Both h1 evacs ride DVE:
                    the ACT queue drifts ~a step behind (its h2 op depends on
                    the late-in-step L2), which made ACT-side h1 arrive
                    just-in-time for L2 and stall its strip LDW."""
                    ps1 = p1.tile([128, TB], f32, tag="mm1", name=f"ps1_{i}")
                    h0prev = h0t[tm][i]
                    for c in range(2):
                        nc.tensor.matmul(ps1[:], W["WE1"][:, ex, c, :],
                                         h0prev[:, c, :], start=(c == 0),
                                         stop=(c == 1))
                    h1_ = h1p.tile([128, TB], bf16, tag="h1", name=f"h1_{i}")
                    if i == 0:
                        dve_evac(h1_[:], ps1[:], W["BE1"][:, ex:ex + 1])
                    else:
                        nc.scalar.activation(h1_[:], ps1[:], Relu,
                                             bias=W["BE1"][:, ex:ex + 1])
                    h1t.setdefault(tm, [None, None])[i] = h1_

                def gating_a(q):
                    psg = p2.tile([128, TB], f32, tag="mm2", name="psg")
                    for c in range(KD):
                        nc.tensor.matmul(psg[:], W["WG1"][:, c, :],
                                         xt[4 * j + q][:, c, :],
                                         start=(c == 0), stop=(c == KD - 1))
                    gh_ = ghp.tile([128, TB], bf16, tag="gh", name=f"gh{q}")
                    dve_evac(gh_[:], psg[:], W["BG1"][:, 0:1])
                    ghq[q] = gh_

                for t in range(NT + 5):
                    if j == 0 and 1 <= t <= 12:
                        load_x(t + 3)
                        if t % 4 == 0:
                            load_pair(t // 4)
                    tm1, tm2, tm3 = t - 1, t - 2, t - 3

                    # gating runs late in each phase (t=12/13, burst 15) so
                    # the eg export is off the kernel-end critical path
                    ga_t = 12
                    if t == ga_t:
                        gating_a(0)
                        gating_a(1)
                    if t == ga_t + 1:
                        gating_a(2)
                        gating_a(3)

                    # L0 for tile t (both experts), L1 for tile t-1 interleaved
                    if t < NT:
                        h0 = [h0p.tile([128, 2, TB], bf16, tag="h0",
                                       name=f"h0_{i}") for i in (0, 1)]
                        for i, ex in ((0, ea), (1, eb)):
                            if fast_h0:
                                ps0 = p0.tile([128, 2, TB], f32, tag="mm0",
                                              name=f"ps0_{i}")
                                for mc in range(2):
                                    for c in range(KD):
                                        nc.tensor.matmul(ps0[:, mc, :],
                                                         W["WE0"][:, ex, c, mc, :],
                                                         xt[t][:, c, :],
                                                         start=(c == 0),
                                                         stop=(c == KD - 1))
                                if i == 0:
                                    nc.vector.tensor_scalar_max(h0[i][:], ps0[:],
                                                                0.0)
                                else:
                                    nc.scalar.activation(h0[i][:], ps0[:], Relu)
                            else:
                                for mc in range(2):
                                    ps0 = p0.tile([128, TB], f32, tag="mm0",
                                                  name=f"ps0_{i}{mc}")
                                    for c in range(KD):
                                        nc.tensor.matmul(ps0[:],
                                                         W["WE0"][:, ex, c, mc, :],
                                                         xt[t][:, c, :],
                                                         start=(c == 0),
                                                         stop=(c == KD - 1))
                                    if mc == 0:
                                        dve_evac(h0[i][:, 0, :], ps0[:],
                                                 W["BE0"][:, ex, 0:1])
                                    else:
                                        nc.scalar.activation(h0[i][:, mc, :],
                                                             ps0[:], Relu,
                                                             bias=W["BE0"][:, ex, mc:mc + 1])
                            if tm1 >= 0:
                                l1_block(tm1, i, ex)
                        h0t[t] = h0
                        if tm1 >= 0:
                            del h0t[tm1]
                    elif t == NT:
                        for i, ex in ((0, ea), (1, eb)):
                            l1_block(tm1, i, ex)
                        del h0t[tm1]

                    # L2 for tiles (t-3, t-2) batched on even steps: the
                    # col-tiled group's entry (strip LDW can't prefetch under
                    # a full-row matmul) and exit (strip drain delays the next
                    # full-array matmul) costs ~217ns and is paid once per
                    # burst instead of once per tile.
                    # even steps cover (t-3, t-2); the extra t=17 slot pulls
                    # tile 15 one step earlier to shorten the drain tail
                    if t % 2 == 0 or t == 17:
                        for tl in (t - 3, t - 2):
                            if not (0 <= tl < NT) or tl not in h1t:
                                continue
                            ps2 = p2.tile([128, TB], f32, tag="mm2", name="ps2")
                            ha, hb = h1t[tl]
                            nc.tensor.matmul(ps2[0:64, :], W["WE2"][:, j, 0, :],
                                             ha[:], start=True, stop=True,
                                             tile_position=(0, 0),
                                             skip_group_check=True)
                            nc.tensor.matmul(ps2[64:128, :], W["WE2"][:, j, 1, :],
                                             hb[:], start=True, stop=True,
                                             tile_position=(0, 64),
                                             skip_group_check=True)
                            h2_ = h2p.tile([128, TB], bf16, tag="h2", name="h2")
                            nc.scalar.activation(h2_[:], ps2[:], Relu,
                                                 bias=W["BE2"][:, j:j + 1])
                            h2t[tl] = h2_
                            del h1t[tl]

                    # Z burst for tiles 4q..4q+3 at t=4q+7 (odd steps, away
                    # from the even-step L2 bursts; clustering them measured
                    # slower - the combined step overloads the evac engines).
                    # The last group fires at t=18 (h2(15) lands at t=17).
                    zq = None
                    if t in (7, 11, 15):
                        zq = (t - 7) // 4
                    elif t == 18:
                        zq = 3
                    if zq is not None:
                        q = zq
                        psz = p2.tile([128, TB], f32, tag="mm2", name="psz")
                        for s in range(4):
                            nc.tensor.matmul(psz[32 * s:32 * s + 32, :],
                                             W["OWZ"][:], h2t[4 * q + s][:],
                                             start=True, stop=True,
                                             tile_position=(0, 32 * s),
                                             skip_group_check=True)
                        zb = zbp.tile([128, TB], f32, tag="zb", name="zb")
                        nc.vector.tensor_scalar_add(zb[:], psz[:], 0.0)
                        if j == NPAIR - 1 and q == NT // 4 - 1:
                            # final export: 4 tiny row-pair DMAs spread over 4
                            # queues land ~3us earlier than one 256KB transfer
                            # keep gpsimd out of the final exports: its
                            # teardown DRAIN waits on the whole DMA ring
                            for s, eng in ((0, nc.sync), (1, nc.scalar),
                                           (2, nc.sync), (3, nc.scalar)):
                                eng.dma_start(zs[j, q, 32 * s:32 * s + 2],
                                              zb[32 * s:32 * s + 2, :])
                        else:
                            nc.sync.dma_start(zs[j, q], zb[:])
                        for s in range(4):
                            del h2t[4 * q + s]

                    # gating L2 burst: 4 col-tiled M=32 (strip s = tile 4j+s)
                    if t == 15:
                        psb = p2.tile([128, TB], f32, tag="mm2", name="psb")
                        for s in range(4):
                            nc.tensor.matmul(psb[32 * s:32 * s + 32, :],
                                             W["WG2"][:], ghq[s][:],
                                             start=True, stop=True,
                                             tile_position=(0, 32 * s),
                                             skip_group_check=True)
                        egb = egp.tile([128, TB], f32, tag="eg", name="egb")
                        nc.scalar.activation(egb[:], psb[:], Exp,
                                             bias=W["BG2"][:, 0:1])
                        nc.gpsimd.dma_start(eg[j], egb[:])

    nc.compile()
    return nc


_CACHE = {}


def _get_program(fast_h0):
    key = ("nc", fast_h0)
    if key not in _CACHE:
        _CACHE[key] = _build_program(fast_h0=fast_h0)
    return _CACHE[key]


def _run(inputs, trace=False):
    from concourse.bass_utils import run_bass_kernel_spmd

    x = np.ascontiguousarray(np.asarray(inputs["x"], dtype=np.float32))
    dev, ob = _fold_params(inputs)
    # fast_h0 measured SLOWER on hw (251us vs 229us): the merged FD=1024 evac
    # releases the two-bank p0 ring later and stalls the next step's L0.
    fast_h0 = False
    nc = _get_program(fast_h0)

    in_maps = []
    for c in range(NCORES):
        m = dict(dev)
        xs = np.ascontiguousarray(x[c * NB:(c + 1) * NB, :].T)
        m["xT"] = xs.astype(ml_dtypes.bfloat16)
        in_maps.append(m)

    kwargs = {}
    if trace:
        kwargs = dict(trace=True, trace_cores=[0])
    res = run_bass_kernel_spmd(nc, in_maps, core_ids=list(range(NCORES)), **kwargs)

    outs = []
    for c in range(NCORES):
        zf = res.results[c]["zs"].astype(np.float64)     # [NPAIR, NT//4, 128, TB]
        gf = res.results[c]["eg"].astype(np.float64)     # [NPAIR, 128, TB]
        # zf[j, q, 32s+i] -> [t=4q+s, e=2j+i, col]
        z = zf.reshape(NPAIR, NT // 4, 4, 32, TB)[:, :, :, 0:2]
        z2 = z.transpose(1, 2, 0, 3, 4).reshape(NT, E, TB)
        # gf[j, 32s+e]: gating tile t=4j+s -> [t, e, col]
        g = gf.reshape(NPAIR, 4, 32, TB)[:, :, 0:E]
        g2 = g.reshape(NT, E, TB)
        num = np.sum(g2 * z2, axis=1)                    # [NT, TB]
        den = np.sum(g2, axis=1)
        outs.append((num / den).reshape(-1))
    out = np.concatenate(outs) + ob
    return out.astype(np.float32)[:, None], res


def kernel(**inputs):
    out, _ = _run(inputs, trace=False)
    return out


def kernel_traced(**inputs):
    return _run(inputs, trace=True)


# revision 64
# speedup vs baseline: 1.0446x; 1.0139x over previous
"""MoE (gating + 8 experts, BN-folded) Trainium2 Bass kernel, v4.

Contract: kernel(**inputs) takes the FULL unsharded inputs (numpy, keyed as in
setup_inputs()) and returns the FULL [65536, 1] float32 output.

Strategy (v4, evolved from v3; ~226us vs v3's 267.8us on core 0):
  * Data-parallel over 8 NeuronCores: batch 65536 -> 8192 rows per core.
  * All BatchNorms folded into the adjacent Linear weights/biases on host.
  * Activations live as [features(partitions), batch(free)]; x is transposed
    host-side per shard and stays resident in SBUF.
  * Software pipeline with a full step of slack between producer and consumer:
    step t runs L0(t) and L1(t-1), so every PSUM evacuation has ~3us before
    its result is needed and the PE sustains the 216 ns/matmul back-to-back
    rate (v3 ran L1(t) in step t and stalled on the evac turnaround).
  * L2 runs as 2-tile bursts on even steps (tiles t-3 and t-2), each tile a
    pair of col-tiled M=64 matmuls (tile_position (0,0)/(0,64)) running
    concurrently on separate array column strips. A col-tiled group pays
    ~217ns of entry/exit transitions against the surrounding full-array
    stream (strip LDWEIGHTS cannot prefetch under a full-row matmul, and the
    strip drain delays the next full-array matmul), so bursts amortize it.
  * Zproj batched per 4 tiles as a 4x col-tiled M=32 burst into one PSUM bank
    at t=4q+7 (strip s = tile 4q+s, rows 32s..32s+1 hold z_a/z_b), one DVE
    evac + one whole-tile DMA per burst.
  * gating runs at t=12/13 with a 4x col-tiled M=32 logit burst at t=15, so
    its export is never on the kernel-end critical path.
  * Evacuation assignment (h0mc0/h1a/zb/gh on DVE; h0mc1/h1b/h2/eg on ACT) is
    tuned so each engine's queue order matches its dependency-fire order;
    per-engine FIFO head-of-line blocking otherwise stalls the PE rings.
  * PSUM budget: p0 4 banks (L0), p1 2 banks (L1), shared 2-bank ring
    (L2 / gating / Z). 8 cold warm-up matmuls span the HAM clock ramp.
  * z and raw gate numerators exp(logits) are exported; the host computes
    y = sum_e g_e z_e / sum_e g_e + ob in float64.
"""

import numpy as np
import ml_dtypes

EPS = 1e-5
B, D, E, G, H0, H1, H2 = 65536, 256, 8, 128, 256, 128, 64
NCORES = 8
NB = B // NCORES          # rows per core
TB = 512                  # batch tile (matmul free dim / PSUM bank)
NT = NB // TB             # batch tiles per core
KD = D // 128             # k-chunks over D
NPAIR = E // 2


def _fold_params(inputs):
    """Fold the four BatchNorms into the adjacent Linears. float64 math."""
    f = {k: np.asarray(v, dtype=np.float64) for k, v in inputs.items()}

    s_in = f["in_g"] / np.sqrt(f["in_v"] + EPS)            # [D]
    t_in = f["in_b"] - f["in_m"] * s_in                    # [D]

    # gating L1 (+input BN folded in)
    a_g = f["g_g"] / np.sqrt(f["g_v"] + EPS)               # [G]
    w1 = f["gW1"] * a_g[None, :]                           # [D,G]
    W1f = s_in[:, None] * w1
    b1f = t_in @ w1 + (f["gb1"] - f["g_m"]) * a_g + f["g_b"]

    # expert L0 (+input BN)
    a0 = f["e0g"] / np.sqrt(f["e0v"] + EPS)                # [E,H0]
    w0 = f["eW0"] * a0[:, None, :]                         # [E,D,H0]
    W0f = s_in[None, :, None] * w0
    b0f = np.einsum("d,edo->eo", t_in, w0) + (f["eb0"] - f["e0m"]) * a0 + f["e0b"]

    a1 = f["e1g"] / np.sqrt(f["e1v"] + EPS)
    W1ef = f["eW1"] * a1[:, None, :]                       # [E,H0,H1]
    b1ef = (f["eb1"] - f["e1m"]) * a1 + f["e1b"]

    a2 = f["e2g"] / np.sqrt(f["e2v"] + EPS)
    W2f = f["eW2"] * a2[:, None, :]                        # [E,H1,H2]
    b2f = (f["eb2"] - f["e2m"]) * a2 + f["e2b"]

    g32 = lambda a: np.ascontiguousarray(a, dtype=np.float32)
    gbf = lambda a: np.ascontiguousarray(a, dtype=np.float32).astype(ml_dtypes.bfloat16)

    dev = {}
    dev["WG1"] = gbf(W1f.reshape(KD, 128, G).transpose(1, 0, 2))          # [128,KD,G]
    dev["BG1"] = g32(b1f[:, None])                                        # [G,1]
    WG2 = np.zeros((G, 32), dtype=np.float64)
    WG2[:, 0:E] = f["gW2"]                                                # M=32 strip
    dev["WG2"] = gbf(WG2)                                                 # [128,32]
    BG2 = np.zeros((128, 1), dtype=np.float64)
    for s in range(4):
        BG2[32 * s:32 * s + E, 0] = f["gb2"]
    dev["BG2"] = g32(BG2)                                                 # [128,1]
    dev["WE0"] = gbf(W0f.reshape(E, KD, 128, 2, 128).transpose(2, 0, 1, 3, 4))  # [128,E,KD,2,128]
    dev["BE0"] = g32(b0f.reshape(E, 2, 128).transpose(2, 0, 1))           # [128,E,2]
    dev["WE1"] = gbf(W1ef.reshape(E, 2, 128, H1).transpose(2, 0, 1, 3))   # [128,E,2,H1]
    dev["BE1"] = g32(b1ef.T)                                              # [H1,E]
    WE2 = np.zeros((128, NPAIR, 2, 64), dtype=np.float64)                 # per-ex strips
    BE2 = np.zeros((128, NPAIR), dtype=np.float64)
    for j in range(NPAIR):
        WE2[:, j, 0, :] = W2f[2 * j]                                      # [H1,64]
        WE2[:, j, 1, :] = W2f[2 * j + 1]
        BE2[0:64, j] = b2f[2 * j]
        BE2[64:128, j] = b2f[2 * j + 1]
    dev["WE2"] = gbf(WE2)
    dev["BE2"] = g32(BE2)
    ow = f["oW"][:, 0]                                                    # [H2]
    OWZ = np.zeros((128, 32), dtype=np.float64)                           # M=32 strip
    OWZ[0:64, 0] = ow
    OWZ[64:128, 1] = ow
    dev["OWZ"] = gbf(OWZ)
    ob = float(f["ob"][0])
    return dev, ob


def _build_program(fast_h0=False):
    """fast_h0: h0 biases are all zero (true for the reference init, where
    every BatchNorm is identity and every Linear bias is zero), so the two
    per-mc h0 evacuations merge into one FD=1024 two-bank op per expert.
    The general path (per-mc bias via tensor_scalar/activation) is kept as
    the fallback for nonzero biases."""
    import concourse.bass as bass
    import concourse.mybir as mybir
    import concourse.tile as tile
    from concourse import bacc

    f32 = mybir.dt.float32
    bf16 = mybir.dt.bfloat16
    Relu = mybir.ActivationFunctionType.Relu
    Exp = mybir.ActivationFunctionType.Exp
    Copy = mybir.ActivationFunctionType.Copy
    add = mybir.AluOpType.add
    amax = mybir.AluOpType.max

    nc = bacc.Bacc("TRN2", target_bir_lowering=False, debug=False)

    xT = nc.dram_tensor("xT", [D, NB], bf16, kind="ExternalInput").ap()
    # zs[j, q, 32s+i, col] = z for expert 2j+i, batch tile 4q+s (full zb tile,
    # one large DMA per burst beats 4 small ones: each DMA_DIRECT2D costs
    # ~600ns on its issuing queue and the tail ones serialized at kernel end)
    zs = nc.dram_tensor("zs", [NPAIR, NT // 4, 128, TB], f32, kind="ExternalOutput").ap()
    # eg[j, 32s+e, col] = exp(logit_e) for batch tile 4j+s
    eg = nc.dram_tensor("eg", [NPAIR, 128, TB], f32, kind="ExternalOutput").ap()

    d_in = {}
    shapes = {
        "WG1": ([128, KD, G], bf16), "BG1": ([G, 1], f32),
        "WG2": ([128, 32], bf16), "BG2": ([128, 1], f32),
        "WE0": ([128, E, KD, 2, 128], bf16), "BE0": ([128, E, 2], f32),
        "WE1": ([128, E, 2, H1], bf16), "BE1": ([H1, E], f32),
        "WE2": ([128, NPAIR, 2, 64], bf16), "BE2": ([128, NPAIR], f32),
        "OWZ": ([128, 32], bf16),
    }
    for name, (shape, dt) in shapes.items():
        d_in[name] = nc.dram_tensor(name, shape, dt, kind="ExternalInput").ap()

    with tile.TileContext(nc) as tc:
        with (
            tc.tile_pool(name="consts", bufs=1) as consts,
            tc.tile_pool(name="xp", bufs=NT) as xp,
            tc.tile_pool(name="ghp", bufs=5) as ghp,
            tc.tile_pool(name="egp", bufs=2) as egp,
            tc.tile_pool(name="h0p", bufs=6) as h0p,
            tc.tile_pool(name="h1p", bufs=8) as h1p,
            tc.tile_pool(name="h2p", bufs=7) as h2p,
            tc.tile_pool(name="zbp", bufs=2) as zbp,
            tc.tile_pool(name="p0", bufs=(2 if fast_h0 else 4),
                         space="PSUM") as p0,                        # 4 banks: L0
            tc.tile_pool(name="p1", bufs=2, space="PSUM") as p1,     # 2 banks: L1
            tc.tile_pool(name="p2", bufs=2, space="PSUM") as p2,     # 2 banks: L2/gate/Z
        ):
            W = {}
            for name, ap in d_in.items():
                W[name] = consts.tile(list(ap.shape), shapes[name][1], tag=name,
                                      name=name)
            # Expert weights stream on the gpsimd queue in usage order; the
            # small biases ride the scalar queue so nothing serializes behind
            # the big transfers.
            def load_pair(j):
                # both experts' L0 weights first: the first L0 step needs
                # WE0 of BOTH experts before any WE1 is touched
                for i in (0, 1):
                    e = 2 * j + i
                    nc.gpsimd.dma_start(W["WE0"][:, e], d_in["WE0"][:, e])
                for i in (0, 1):
                    e = 2 * j + i
                    nc.gpsimd.dma_start(W["WE1"][:, e], d_in["WE1"][:, e])
                nc.gpsimd.dma_start(W["WE2"][:, j], d_in["WE2"][:, j])

            # warm-tile memset FIRST on the gpsimd queue (before load_pair's
            # DMAs land on it) so the PE warm-up can start as soon as the
            # engines leave the NEFF preamble.
            warm = consts.tile([128, TB], bf16, tag="warm", name="warm")
            nc.gpsimd.memset(warm[:], 0.0)

            load_pair(0)
            for name in ("BE0", "BE1", "WG1", "BG1", "BE2", "WG2", "BG2", "OWZ"):
                nc.scalar.dma_start(W[name][:], d_in[name][:])

            # first 4 x tiles up-front; the rest stream during phase 0 so the
            # early DMA bandwidth goes to the pair-0 weights.
            xt = []

            def load_x(t):
                bs = t * TB
                xti = xp.tile([128, KD, TB], bf16, tag="xt", name=f"x{t}")
                for c in range(KD):
                    nc.sync.dma_start(xti[:, c, :], xT[c * 128:(c + 1) * 128, bs:bs + TB])
                xt.append(xti)

            for t in range(4):
                load_x(t)

            # PE warm-up: the HAM clock gate needs ~3.4us of sustained matmul
            # activity to lift the PE from 1.2 to 2.4 GHz; burn the DMA fill
            # time on dummy matmuls over a zeroed tile so the real matmuls
            # start warm.
            # PE warm-up: 8 cold matmuls (427ns each) span exactly the ~3.4us
            # HAM clock-gate ramp, so the first real L0 runs at 2.4 GHz.
            # (warm=3 measured slower: the cold-clock L0s delay their evacs
            # and the stall ripples through the pipeline.)
            for k in range(8):
                pool = (p1, p2)[k % 2]
                tag = "mm1" if k % 2 == 0 else "mm2"
                psw = pool.tile([128, TB], f32, tag=tag, name=f"warm{k}")
                nc.tensor.matmul(psw[:], warm[:, 0:128], warm[:],
                                 start=True, stop=True)

            def dve_evac(dst, src, bias_ap):
                nc.vector.tensor_scalar(out=dst, in0=src, scalar1=bias_ap,
                                        scalar2=0.0, op0=add, op1=amax)

            # ---- expert pair phases ----
            for j in range(NPAIR):
                ea, eb = 2 * j, 2 * j + 1
                h0t = {}
                h1t = {}
                h2t = {}
                ghq = {}

                def l1_block(tm, i, ex):
                    """L1 for tile tm, expert slot i. Both h1 evacs ride DVE:
                    the ACT queue drifts ~a step behind (its h2 op depends on
                    the late-in-step L2), which made ACT-side h1 arrive
                    just-in-time for L2 and stall its strip LDW."""
                    ps1 = p1.tile([128, TB], f32, tag="mm1", name=f"ps1_{i}")
                    h0prev = h0t[tm][i]
                    for c in range(2):
                        nc.tensor.matmul(ps1[:], W["WE1"][:, ex, c, :],
                                         h0prev[:, c, :], start=(c == 0),
                                         stop=(c == 1))
                    h1_ = h1p.tile([128, TB], bf16, tag="h1", name=f"h1_{i}")
                    if i == 0:
                        dve_evac(h1_[:], ps1[:], W["BE1"][:, ex:ex + 1])
                    else:
                        nc.scalar.activation(h1_[:], ps1[:], Relu,
                                             bias=W["BE1"][:, ex:ex + 1])
                    h1t.setdefault(tm, [None, None])[i] = h1_

                def gating_a(q):
                    psg = p2.tile([128, TB], f32, tag="mm2", name="psg")
                    for c in range(KD):
                        nc.tensor.matmul(psg[:], W["WG1"][:, c, :],
                                         xt[4 * j + q][:, c, :],
                                         start=(c == 0), stop=(c == KD - 1))
                    gh_ = ghp.tile([128, TB], bf16, tag="gh", name=f"gh{q}")
                    dve_evac(gh_[:], psg[:], W["BG1"][:, 0:1])
                    ghq[q] = gh_

                for t in range(NT + 5):
                    if j == 0 and 1 <= t <= 12:
                        load_x(t + 3)
                        if t % 4 == 0:
                            load_pair(t // 4)
                    tm1, tm2, tm3 = t - 1, t - 2, t - 3

                    # gating runs late in each phase (t=12/13, burst 15) so
                    # the eg export is off the kernel-end critical path
                    ga_t = 12
                    if t == ga_t:
                        gating_a(0)
                        gating_a(1)
                    if t == ga_t + 1:
                        gating_a(2)
                        gating_a(3)

                    # L0 for tile t (both experts), L1 for tile t-1 interleaved
                    if t < NT:
                        h0 = [h0p.tile([128, 2, TB], bf16, tag="h0",
                                       name=f"h0_{i}") for i in (0, 1)]
                        for i, ex in ((0, ea), (1, eb)):
                            if fast_h0:
                                ps0 = p0.tile([128, 2, TB], f32, tag="mm0",
                                              name=f"ps0_{i}")
                                for mc in range(2):
                                    for c in range(KD):
                                        nc.tensor.matmul(ps0[:, mc, :],
                                                         W["WE0"][:, ex, c, mc, :],
                                                         xt[t][:, c, :],
                                                         start=(c == 0),
                                                         stop=(c == KD - 1))
                                if i == 0:
                                    nc.vector.tensor_scalar_max(h0[i][:], ps0[:],
                                                                0.0)
                                else:
                                    nc.scalar.activation(h0[i][:], ps0[:], Relu)
                            else:
                                for mc in range(2):
                                    ps0 = p0.tile([128, TB], f32, tag="mm0",
                                                  name=f"ps0_{i}{mc}")
                                    for c in range(KD):
                                        nc.tensor.matmul(ps0[:],
                                                         W["WE0"][:, ex, c, mc, :],
                                                         xt[t][:, c, :],
                                                         start=(c == 0),
                                                         stop=(c == KD - 1))
                                    if mc == 0:
                                        dve_evac(h0[i][:, 0, :], ps0[:],
                                                 W["BE0"][:, ex, 0:1])
                                    else:
                                        nc.scalar.activation(h0[i][:, mc, :],
                                                             ps0[:], Relu,
                                                             bias=W["BE0"][:, ex, mc:mc + 1])
                            if tm1 >= 0:
                                l1_block(tm1, i, ex)
                        h0t[t] = h0
                        if tm1 >= 0:
                            del h0t[tm1]
                    elif t == NT:
                        for i, ex in ((0, ea), (1, eb)):
                            l1_block(tm1, i, ex)
                        del h0t[tm1]

                    # L2 for tiles (t-3, t-2) batched on even steps: the
                    # col-tiled group's entry (strip LDW can't prefetch under
                    # a full-row matmul) and exit (strip drain delays the next
                    # full-array matmul) costs ~217ns and is paid once per
                    # burst instead of once per tile.
                    # even steps cover (t-3, t-2); the extra t=17 slot pulls
                    # tile 15 one step earlier to shorten the drain tail
                    if t % 2 == 0 or t == 17:
                        for tl in (t - 3, t - 2):
                            if not (0 <= tl < NT) or tl not in h1t:
                                continue
                            ps2 = p2.tile([128, TB], f32, tag="mm2", name="ps2")
                            ha, hb = h1t[tl]
                            nc.tensor.matmul(ps2[0:64, :], W["WE2"][:, j, 0, :],
                                             ha[:], start=True, stop=True,
                                             tile_position=(0, 0),
                                             skip_group_check=True)
                            nc.tensor.matmul(ps2[64:128, :], W["WE2"][:, j, 1, :],
                                             hb[:], start=True, stop=True,
                                             tile_position=(0, 64),
                                             skip_group_check=True)
                            h2_ = h2p.tile([128, TB], bf16, tag="h2", name="h2")
                            # split the burst's two h2 evacs across engines:
                            # five ACT ops exceed the even-step budget, and
                            # the DVE-side op is last in its queue, matching
                            # dependency-fire order
                            if tl == t - 3:
                                dve_evac(h2_[:], ps2[:], W["BE2"][:, j:j + 1])
                            else:
                                nc.scalar.activation(h2_[:], ps2[:], Relu,
                                                     bias=W["BE2"][:, j:j + 1])
                            h2t[tl] = h2_
                            del h1t[tl]

                    # Z burst for tiles 4q..4q+3 at t=4q+7 (odd steps, away
                    # from the even-step L2 bursts; clustering them measured
                    # slower - the combined step overloads the evac engines).
                    # The last group fires at t=18 (h2(15) lands at t=17).
                    zq = None
                    if t in (7, 11, 15):
                        zq = (t - 7) // 4
                    elif t == 18:
                        zq = 3
                    if zq is not None:
                        q = zq
                        psz = p2.tile([128, TB], f32, tag="mm2", name="psz")
                        for s in range(4):
                            nc.tensor.matmul(psz[32 * s:32 * s + 32, :],
                                             W["OWZ"][:], h2t[4 * q + s][:],
                                             start=True, stop=True,
                                             tile_position=(0, 32 * s),
                                             skip_group_check=True)
                        zb = zbp.tile([128, TB], f32, tag="zb", name="zb")
                        nc.vector.tensor_scalar_add(zb[:], psz[:], 0.0)
                        if j == NPAIR - 1 and q == NT // 4 - 1:
                            # final export: 4 tiny row-pair DMAs spread over 4
                            # queues land ~3us earlier than one 256KB transfer
                            # keep gpsimd out of the final exports: its
                            # teardown DRAIN waits on the whole DMA ring
                            for s, eng in ((0, nc.sync), (1, nc.scalar),
                                           (2, nc.sync), (3, nc.scalar)):
                                eng.dma_start(zs[j, q, 32 * s:32 * s + 2],
                                              zb[32 * s:32 * s + 2, :])
                        else:
                            nc.sync.dma_start(zs[j, q], zb[:])
                        for s in range(4):
                            del h2t[4 * q + s]

                    # gating L2 burst: 4 col-tiled M=32 (strip s = tile 4j+s)
                    if t == 15:
                        psb = p2.tile([128, TB], f32, tag="mm2", name="psb")
                        for s in range(4):
                            nc.tensor.matmul(psb[32 * s:32 * s + 32, :],
                                             W["WG2"][:], ghq[s][:],
                                             start=True, stop=True,
                                             tile_position=(0, 32 * s),
                                             skip_group_check=True)
                        egb = egp.tile([128, TB], f32, tag="eg", name="egb")
                        nc.scalar.activation(egb[:], psb[:], Exp,
                                             bias=W["BG2"][:, 0:1])
                        nc.gpsimd.dma_start(eg[j], egb[:])

    nc.compile()
    return nc


_CACHE = {}


def _get_program(fast_h0):
    key = ("nc", fast_h0)
    if key not in _CACHE:
        _CACHE[key] = _build_program(fast_h0=fast_h0)
    return _CACHE[key]


def _run(inputs, trace=False):
    from concourse.bass_utils import run_bass_kernel_spmd

    x = np.ascontiguousarray(np.asarray(inputs["x"], dtype=np.float32))
    dev, ob = _fold_params(inputs)
    # fast_h0 measured SLOWER on hw (251us vs 229us): the merged FD=1024 evac
    # releases the two-bank p0 ring later and stalls the next step's L0.
    fast_h0 = False
    nc = _get_program(fast_h0)

    in_maps = []
    for c in range(NCORES):
        m = dict(dev)
        xs = np.ascontiguousarray(x[c * NB:(c + 1) * NB, :].T)
        m["xT"] = xs.astype(ml_dtypes.bfloat16)
        in_maps.append(m)

    kwargs = {}
    if trace:
        kwargs = dict(trace=True, trace_cores=[0])
    res = run_bass_kernel_spmd(nc, in_maps, core_ids=list(range(NCORES)), **kwargs)

    outs = []
    for c in range(NCORES):
        zf = res.results[c]["zs"].astype(np.float64)     # [NPAIR, NT//4, 128, TB]
        gf = res.results[c]["eg"].astype(np.float64)     # [NPAIR, 128, TB]
        # zf[j, q, 32s+i] -> [t=4q+s, e=2j+i, col]
        z = zf.reshape(NPAIR, NT // 4, 4, 32, TB)[:, :, :, 0:2]
        z2 = z.transpose(1, 2, 0, 3, 4).reshape(NT, E, TB)
        # gf[j, 32s+e]: gating tile t=4j+s -> [t, e, col]
        g = gf.reshape(NPAIR, 4, 32, TB)[:, :, 0:E]
        g2 = g.reshape(NT, E, TB)
        num = np.sum(g2 * z2, axis=1)                    # [NT, TB]
        den = np.sum(g2, axis=1)
        outs.append((num / den).reshape(-1))
    out = np.concatenate(outs) + ob
    return out.astype(np.float32)[:, None], res


def kernel(**inputs):
    out, _ = _run(inputs, trace=False)
    return out


def kernel_traced(**inputs):
    return _run(inputs, trace=True)
